# revision 1
# baseline (speedup 1.0000x reference)
"""Trainium2 Bass kernel for channel-wise ("transposed") attention.

Reference computation (per batch b, X = x_in[b] reshaped [N=16384, C=256]):
    Q = X Wq ; K = X Wk ; V = X Wv            (columns l2-normalized over tokens for Q,K)
    attn[h,i,j] = softmax_j( qhat_i . khat_j * rescale[h] )   (32x32 per head)
    out = (A_bd @ V^T)^T Wp + bp

Algebraic reduction used here (validated vs reference, rel err ~3e-6):
    S    = X^T X                      [256,256]   (only pass-1 reduction needed)
    P1   = S Wq ; P2 = S Wk
    G    = Wk^T P1                    (raw cross-gram K^T Q)
    nq2  = diag(Wq^T P1) ; nk2 = diag(Wk^T P2)
    L    = G * rk[i] * (rq*rescale_expanded)[j] ;  A = blockdiag-softmax_j(exp(L))
    Wbig = Wv @ (A_bd^T Wp)           [256,256]
    out  = X @ Wbig + bp

So the kernel is two streaming passes over X (16.8 MB in / 16.8 MB out per
core) plus tiny 256x256 matmul chains in between.  Each of the 8 cores
processes one batch (data parallel, no collectives).
"""

import sys

if "/opt/trn_rl_repo" not in sys.path:
    sys.path.insert(0, "/opt/trn_rl_repo")

from contextlib import ExitStack

import numpy as np

import concourse.bass as bass
import concourse.tile as tile
from concourse import bacc, mybir
from concourse import bass_utils
from concourse.bass import ds, ts
from concourse.bass_interp import get_hw_module
from concourse.masks import make_identity

F32 = mybir.dt.float32
F32R = mybir.dt.float32r    # PE fast-fp32 (TF32-like, ~1.5e-4 rel); 4x faster N>=256
ALU = mybir.AluOpType
ACTF = mybir.ActivationFunctionType
PSUM = bass.MemorySpace.PSUM

N_CORES = 8
B, H, W, C = 8, 128, 128, 256
HEADS, DH = 8, 32
N = H * W            # 16384 tokens per batch
P = 128              # partitions / token tile
NT = N // P          # 128 token tiles
DMA_TILES = 8        # token tiles per DMA (1 MiB chunks)
NCHUNK = C // P      # 2 channel chunks


def _build_kernel(nc: bacc.Bacc):
    x_dram = nc.dram_tensor("x_in", [N, C], F32, kind="ExternalInput").ap()
    wq_dram = nc.dram_tensor("Wq", [C, C], F32, kind="ExternalInput").ap()
    wk_dram = nc.dram_tensor("Wk", [C, C], F32, kind="ExternalInput").ap()
    wv_dram = nc.dram_tensor("Wv", [C, C], F32, kind="ExternalInput").ap()
    resc_dram = nc.dram_tensor("rescale", [HEADS, 1, 1], F32, kind="ExternalInput").ap()
    wp_dram = nc.dram_tensor("Wp", [C, C], F32, kind="ExternalInput").ap()
    bp_dram = nc.dram_tensor("bp", [C], F32, kind="ExternalInput").ap()
    out_dram = nc.dram_tensor("out", [N, C], F32, kind="ExternalOutput").ap()

    with tile.TileContext(nc) as tc, ExitStack() as top:
        consts = top.enter_context(tc.tile_pool(name="consts", bufs=1))
        xt_pool = top.enter_context(tc.tile_pool(name="xt", bufs=1))
        s_pool = top.enter_context(tc.tile_pool(name="spsum", bufs=1, space=PSUM))

        # ------------- const tiles (instructions emitted inside pass-1 g==0) -------------
        identity_f = consts.tile([P, P], F32)
        identity = consts.tile([P, P], F32R)
        p8 = consts.tile([HEADS, C], F32)        # p8[h,c] = 1 iff c//32 == h
        p8_r = consts.tile([HEADS, C], F32R)
        bdmask = consts.tile([P, NCHUNK, C], F32)  # block-diag head mask chunks
        ones_col_f = consts.tile([P, 1], F32)
        ones_col = consts.tile([P, 1], F32R)     # [128,1] ones: column-sum matmuls
        ones_row = consts.tile([1, P], F32)      # [1,128] ones: partition broadcast
        ones_row_r = consts.tile([1, P], F32R)
        d11 = consts.tile([1, 1], F32)           # ACT table prewarm scratch

        # weight tiles (DMAs issued after the x loads to keep x at queue head)
        wqk = consts.tile([P, NCHUNK, 2 * C], F32)       # [Wq | Wk] row chunks
        wp_sb = consts.tile([P, NCHUNK, C], F32)
        wv_sb = consts.tile([P, NCHUNK, C], F32)
        wvT = consts.tile([P, NCHUNK, C], F32R)          # wvT[p,k,c] = Wv[c, 128k+p]
        wqk_r = consts.tile([P, NCHUNK, 2 * C], F32R)    # rounded copies for f32r mms
        wp_r = consts.tile([P, NCHUNK, C], F32R)
        bp_sb = consts.tile([1, C], F32)
        resc_p = consts.tile([HEADS, 1], F32)
        resc_r = consts.tile([HEADS, 1], F32R)
        bp_r = consts.tile([1, C], F32R)         # rounded bias row (K=1 matmul)
        wbig0 = consts.tile([P, C], F32R)
        wbig1 = consts.tile([P, C], F32R)
        wbig_l = [wbig0, wbig1]

        xT = xt_pool.tile([P, NCHUNK, N], F32R)  # X^T (f32r-rounded), from pass 1

        s_ps0 = s_pool.tile([P, C], F32, space=PSUM)
        s_ps1 = s_pool.tile([P, C], F32, space=PSUM)
        s_ps = [s_ps0, s_ps1]

        # ---------------- pass 1: S = X^T X, and X^T via PE ----------------
        with tc.tile_pool(name="tp", bufs=6, space=PSUM) as tp_pool, tc.tile_pool(
            name="xload", bufs=4
        ) as xload:
            for g in range(NT // DMA_TILES):
                xr = xload.tile([P, DMA_TILES, C], F32R, tag="xr")
                # casting DMA: loads fp32 from HBM, rounds to f32r in-flight
                if g == 0:
                    # small first piece so PE starts sooner
                    for lo, n_t in ((0, 2), (2, 6)):
                        nc.gpsimd.dma_start(
                            xr[:, ds(lo, n_t), :],
                            x_dram[ds((g * DMA_TILES + lo) * P, n_t * P), :].rearrange(
                                "(a p) c -> p a c", p=P
                            ),
                        )
                else:
                    nc.gpsimd.dma_start(
                        xr[:],
                        x_dram[ds(g * DMA_TILES * P, DMA_TILES * P), :].rearrange(
                            "(a p) c -> p a c", p=P
                        ),
                    )
                if g == 0:
                    # masks / identity (gpsimd) — behind chunk0's descriptor gen
                    make_identity(nc, identity_f[:])
                    nc.vector.tensor_copy(identity[:], identity_f[:])
                    nc.gpsimd.memset(p8[:], 0.0)
                    nc.gpsimd.affine_select(
                        out=p8[:].rearrange("p (b i) -> p b i", i=DH),
                        in_=p8[:].rearrange("p (b i) -> p b i", i=DH),
                        compare_op=ALU.not_equal,
                        fill=1.0,
                        base=0,
                        pattern=[[-1, HEADS], [0, DH]],
                        channel_multiplier=1,
                    )
                    nc.vector.tensor_copy(p8_r[:], p8[:])
                    nc.gpsimd.memset(bdmask[:], 0.0)
                    for r in range(NCHUNK):
                        for a2 in range(P // DH):
                            nc.gpsimd.memset(
                                bdmask[ts(a2, DH), r, ds(r * P + a2 * DH, DH)], 1.0
                            )
                    nc.gpsimd.memset(ones_col_f[:], 1.0)
                    nc.vector.tensor_copy(ones_col[:], ones_col_f[:])
                    nc.gpsimd.memset(ones_row[:], 1.0)
                    nc.vector.tensor_copy(ones_row_r[:], ones_row[:])
                    # prewarm ACT sqrt table set (off critical path)
                    nc.scalar.activation(d11[:], ones_row[:, 0:1], ACTF.Sqrt)
                if g == 1:
                    # weight/bias loads + prep: issued behind the first x chunk
                    for k in range(NCHUNK):
                        nc.sync.dma_start(wqk[:, k, 0:C], wq_dram[ts(k, P), :])
                        nc.sync.dma_start(wqk[:, k, C : 2 * C], wk_dram[ts(k, P), :])
                        nc.sync.dma_start(wp_sb[:, k, :], wp_dram[ts(k, P), :])
                        nc.sync.dma_start(wv_sb[:, k, :], wv_dram[ts(k, P), :])
                    nc.sync.dma_start(bp_sb[:], bp_dram.rearrange("(a c) -> a c", a=1))
                    nc.sync.dma_start(resc_p[:], resc_dram.rearrange("h a b -> h (a b)"))
                    for k in range(NCHUNK):
                        nc.vector.tensor_copy(wqk_r[:, k, :], wqk[:, k, :])
                        nc.vector.tensor_copy(wp_r[:, k, :], wp_sb[:, k, :])
                    nc.vector.tensor_copy(bp_r[:], bp_sb[:])
                    nc.vector.tensor_copy(resc_r[:], resc_p[:])
                    for k in range(NCHUNK):
                        for m in range(NCHUNK):
                            tpv = tp_pool.tile([P, P], F32, space=PSUM, tag="tp")
                            nc.tensor.transpose(
                                tpv[:].bitcast(F32), wv_sb[:, m, ts(k, P)], identity_f[:]
                            )
                            nc.vector.tensor_copy(wvT[:, k, ts(m, P)], tpv[:].bitcast(F32))
                for a in range(DMA_TILES):
                    t = g * DMA_TILES + a
                    x_t = xr[:, a, :]
                    first, last = t == 0, t == NT - 1
                    # both chunk transposes land in ONE psum bank (disjoint
                    # column halves); a single strided eviction then writes
                    # both xT chunks -> half the eviction ops on DVE/ACT
                    tp = tp_pool.tile([P, 2 * P], F32R, space=PSUM, tag="tp")
                    for k in range(NCHUNK):
                        nc.tensor.matmul(
                            s_ps[k][:],
                            x_t[:, ts(k, P)],
                            x_t[:],
                            start=first,
                            stop=last,
                        )
                        nc.tensor.transpose(tp[:, ts(k, P)], x_t[:, ts(k, P)], identity[:])
                    tp_v = tp[:].rearrange("p (k c) -> p k c", k=NCHUNK)
                    if t % 2 == 0:
                        nc.vector.tensor_copy(xT[:, :, ts(t, P)], tp_v)
                    else:
                        nc.scalar.copy(xT[:, :, ts(t, P)], tp_v)

        # ---------------- phase B: 256x256 attention math ----------------
        # All intermediates are split into per-chunk tensors: Tile tracks
        # dependencies per tensor, so chunk-0 consumers would otherwise wait
        # for chunk-1 writes of a shared tensor.
        with tc.tile_pool(name="bwork", bufs=3, space=PSUM) as bwork, tc.tile_pool(
            name="bsmall", bufs=2, space=PSUM
        ) as bsmall, tc.tile_pool(name="bsb", bufs=1) as bsb:
            # re-warm the Sqrt table NOW: phase-A Copy activations may have
            # swapped the set; this dummy has no data deps, so its table load
            # overlaps the S-copy -> P12 -> GK window instead of stalling rk
            nc.scalar.activation(d11[:], ones_row[:, 0:1], ACTF.Sqrt)
            s_sbl, p12_psl, p12_sbl, gkl = [], [], [], []
            for k in range(NCHUNK):
                s_k = bsb.tile([P, C], F32R, name=f"s_sb{k}", tag="ssb", bufs=2)
                nc.vector.tensor_copy(s_k[:], s_ps[k][:])
                s_sbl.append(s_k)

            # P12 = S @ [Wq | Wk]   (uses S symmetric: lhsT = S chunks)
            for m in range(NCHUNK):
                pp = bwork.tile([P, 2 * C], F32, space=PSUM, name=f"p12ps{m}", tag="bw", bufs=3)
                for k in range(NCHUNK):
                    nc.tensor.matmul(
                        pp[:],
                        s_sbl[k][:, ts(m, P)],
                        wqk_r[:, k, :],
                        start=(k == 0),
                        stop=(k == 1),
                    )
                p12_psl.append(pp)
            for m in range(NCHUNK):
                psb = bsb.tile([P, 2 * C], F32R, name=f"p12sb{m}", tag="p12sb", bufs=2)
                nc.vector.tensor_copy(psb[:], p12_psl[m][:])
                p12_sbl.append(psb)

            # [G | Kgram] = Wk^T @ [P1 | P2]
            for m in range(NCHUNK):
                gg = bwork.tile([P, 2 * C], F32, space=PSUM, name=f"gkps{m}", tag="bw", bufs=3)
                for k in range(NCHUNK):
                    nc.tensor.matmul(
                        gg[:],
                        wqk_r[:, k, ds(C + m * P, P)],
                        p12_sbl[k][:],
                        start=(k == 0),
                        stop=(k == 1),
                    )
                gkl.append(gg)

            # nq2[j] = sum_c Wq[c,j] P1[c,j]  -> [1, 256] via ones-matmul
            qpl = []
            for k in range(NCHUNK):
                qp = bsb.tile([P, C], F32R, name=f"qp{k}", tag="qp", bufs=2)
                nc.vector.tensor_mul(
                    qp[:],
                    wqk_r[:, k, 0:C].bitcast(F32),
                    p12_sbl[k][:, 0:C].bitcast(F32),
                )
                qpl.append(qp)
            nq2_ps = bsmall.tile([1, C], F32, space=PSUM, tag="bs")
            for k in range(NCHUNK):
                nc.tensor.matmul(
                    nq2_ps[:], ones_col[:], qpl[k][:], start=(k == 0), stop=(k == 1)
                )

            # nk2 rows: diag of Kgram chunk m  -> per-partition [128,1]
            nk2 = bsb.tile([P, NCHUNK], F32)
            scrap = bsb.tile([P, P], F32)
            for m in range(NCHUNK):
                nc.vector.scalar_tensor_tensor(
                    out=scrap[:],
                    in0=gkl[m][:, ds(C + m * P, P)],
                    scalar=1.0,
                    in1=identity_f[:],
                    op0=ALU.mult,
                    op1=ALU.mult,
                    accum_out=nk2[:, m : m + 1],
                )
            nk = bsb.tile([P, NCHUNK], F32)
            nc.scalar.activation(nk[:], nk2[:], ACTF.Sqrt)
            rk = bsb.tile([P, NCHUNK], F32)
            nc.vector.reciprocal(rk[:], nk[:])

            # column scale: rq[j] * rescale[head(j)]
            nq = bsb.tile([1, C], F32)
            nc.scalar.activation(nq[:], nq2_ps[:], ACTF.Sqrt)
            rq = bsb.tile([1, C], F32)
            nc.vector.reciprocal(rq[:], nq[:])
            # dummy exp: pulls the Exp table load off the critical path
            nc.scalar.activation(d11[:], ones_row[:, 0:1], ACTF.Exp)
            rexp_ps = bsmall.tile([1, C], F32, space=PSUM, tag="bs")
            nc.tensor.matmul(rexp_ps[:], resc_r[:], p8_r[:])
            colscale = bsb.tile([1, C], F32R)
            nc.vector.tensor_mul(colscale[:], rq[:], rexp_ps[:])
            csbc_ps = bsmall.tile([P, C], F32, space=PSUM, tag="bs")
            nc.tensor.matmul(csbc_ps[:], ones_row_r[:], colscale[:])
            csbc_sb = bsb.tile([P, C], F32)
            nc.vector.tensor_copy(csbc_sb[:], csbc_ps[:])

            # logits -> exp -> masked softmax -> A (block-diagonal elsewhere 0)
            al = []
            for m in range(NCHUNK):
                sc = bsb.tile([P, C], F32, name=f"sc{m}", tag="sc", bufs=2)
                nc.vector.scalar_tensor_tensor(
                    out=sc[:],
                    in0=gkl[m][:, 0:C],
                    scalar=rk[:, m : m + 1],
                    in1=csbc_sb[:],
                    op0=ALU.mult,
                    op1=ALU.mult,
                )
                e = bsb.tile([P, C], F32, name=f"e{m}", tag="e", bufs=2)
                nc.scalar.activation(e[:], sc[:], ACTF.Exp)
                em = bsb.tile([P, C], F32, name=f"em{m}", tag="em", bufs=2)
                den = bsb.tile([P, 1], F32, name=f"den{m}", tag="den", bufs=2)
                nc.vector.scalar_tensor_tensor(
                    out=em[:],
                    in0=e[:],
                    scalar=1.0,
                    in1=bdmask[:, m, :],
                    op0=ALU.mult,
                    op1=ALU.mult,
                    accum_out=den[:],
                )
                rden = bsb.tile([P, 1], F32, name=f"rden{m}", tag="rden", bufs=2)
                nc.vector.reciprocal(rden[:], den[:])
                a_m = bsb.tile([P, C], F32R, name=f"a{m}", tag="a", bufs=2)
                nc.vector.tensor_scalar_mul(a_m[:], em[:], rden[:])
                al.append(a_m)

            # T1 = A_bd^T @ Wp  (lhsT = A_bd chunks directly)
            t1_sbl = []
            for m in range(NCHUNK):
                t1p = bwork.tile([P, C], F32, space=PSUM, name=f"t1ps{m}", tag="bw", bufs=3)
                for k in range(NCHUNK):
                    nc.tensor.matmul(
                        t1p[:],
                        al[k][:, ts(m, P)],
                        wp_r[:, k, :],
                        start=(k == 0),
                        stop=(k == 1),
                    )
                t1s = bsb.tile([P, C], F32R, name=f"t1sb{m}", tag="t1sb", bufs=2)
                nc.vector.tensor_copy(t1s[:], t1p[:])
                t1_sbl.append(t1s)

            # Wbig = Wv @ T1  (lhsT = Wv^T chunks)
            for m in range(NCHUNK):
                wbp = bwork.tile([P, C], F32, space=PSUM, name=f"wbps{m}", tag="bw", bufs=3)
                for k in range(NCHUNK):
                    nc.tensor.matmul(
                        wbp[:],
                        wvT[:, k, ts(m, P)],
                        t1_sbl[k][:],
                        start=(k == 0),
                        stop=(k == 1),
                    )
                nc.vector.tensor_copy(wbig_l[m][:], wbp[:])

        # ---------------- pass 2: out = X @ Wbig + bp ----------------
        OUT_TILES = 8
        with tc.tile_pool(name="ops", bufs=6, space=PSUM) as ops, tc.tile_pool(
            name="outb", bufs=3
        ) as outb:
            for g in range(NT // OUT_TILES):
                ob = outb.tile([P, OUT_TILES, C], F32)
                for a2 in range(OUT_TILES // 2):
                    # two tiles' outputs share one PSUM bank (sequential
                    # accumulation groups in disjoint halves); one strided
                    # eviction writes both -> half the DVE/ACT ops
                    o_ps = ops.tile([P, 2 * C], F32, space=PSUM, tag="o")
                    for h2 in range(2):
                        t = g * OUT_TILES + a2 * 2 + h2
                        for k in range(NCHUNK):
                            nc.tensor.matmul(
                                o_ps[:, ts(h2, C)],
                                xT[:, k, ts(t, P)],
                                wbig_l[k][:],
                                start=(k == 0),
                                stop=False,
                            )
                        nc.tensor.matmul(
                            o_ps[:, ts(h2, C)],
                            ones_row_r[:],
                            bp_r[:],
                            start=False,
                            stop=True,
                        )
                    o_v = o_ps[:].rearrange("p (h c) -> p h c", h=2)
                    if a2 % 2 == 0:
                        nc.vector.tensor_copy(ob[:, ds(a2 * 2, 2), :], o_v)
                    else:
                        nc.scalar.copy(ob[:, ds(a2 * 2, 2), :], o_v)
                if g == NT // OUT_TILES - 1:
                    half = OUT_TILES // 2
                    for h2 in range(2):
                        nc.sync.dma_start(
                            out_dram[
                                ds((g * OUT_TILES + h2 * half) * P, half * P), :
                            ].rearrange("(a p) c -> p a c", p=P),
                            ob[:, ts(h2, half), :],
                        )
                else:
                    nc.sync.dma_start(
                        out_dram[ds(g * OUT_TILES * P, OUT_TILES * P), :].rearrange(
                            "(a p) c -> p a c", p=P
                        ),
                        ob[:],
                    )

    return nc


_NC_CACHE = None


def _get_nc():
    global _NC_CACHE
    if _NC_CACHE is None:
        nc = bacc.Bacc(
            "TRN2",
            target_bir_lowering=False,
            debug=False,
            enable_asserts=False,
            num_devices=N_CORES,
        )
        _build_kernel(nc)
        nc.compile()
        nc.m = get_hw_module(nc.m)
        _NC_CACHE = nc
    return _NC_CACHE


def _make_in_maps(x_in, Wq, Wk, Wv, rescale, Wp, bp):
    x_in = np.ascontiguousarray(np.asarray(x_in, dtype=np.float32))
    maps = []
    for core in range(N_CORES):
        maps.append(
            {
                "x_in": x_in[core].reshape(N, C),
                "Wq": np.asarray(Wq, np.float32),
                "Wk": np.asarray(Wk, np.float32),
                "Wv": np.asarray(Wv, np.float32),
                "rescale": np.asarray(rescale, np.float32),
                "Wp": np.asarray(Wp, np.float32),
                "bp": np.asarray(bp, np.float32),
            }
        )
    return maps


def run_on_hw(inputs: dict, trace: bool = False, tmpdir: str | None = None):
    """Returns (full_output [8,128,128,256] f32, BassKernelResults)."""
    nc = _get_nc()
    in_maps = _make_in_maps(**inputs)
    res = bass_utils.run_bass_kernel_spmd(
        nc, in_maps, core_ids=list(range(N_CORES)), trace=trace, tmpdir=tmpdir
    )
    out = np.stack([res.results[c]["out"].reshape(H, W, C) for c in range(N_CORES)])
    return out.astype(np.float32), res


def kernel(x_in, Wq, Wk, Wv, rescale, Wp, bp) -> np.ndarray:
    out, _ = run_on_hw(
        dict(x_in=x_in, Wq=Wq, Wk=Wk, Wv=Wv, rescale=rescale, Wp=Wp, bp=bp)
    )
    return out



# revision 24
# speedup vs baseline: 1.0014x; 1.0014x over previous
"""Trainium2 Bass kernel for channel-wise ("transposed") attention.

Reference computation (per batch b, X = x_in[b] reshaped [N=16384, C=256]):
    Q = X Wq ; K = X Wk ; V = X Wv            (columns l2-normalized over tokens for Q,K)
    attn[h,i,j] = softmax_j( qhat_i . khat_j * rescale[h] )   (32x32 per head)
    out = (A_bd @ V^T)^T Wp + bp

Algebraic reduction (validated vs reference):
    S    = X^T X                      [256,256]   (only pass-1 reduction needed)
    P1   = S Wq ; P2 = S Wk
    G    = Wk^T P1                    (raw cross-gram K^T Q)
    nq2  = diag(Wq^T P1) ; nk2 = diag(Wk^T P2)
    L    = G * rk[i] * (rq*rescale_expanded)[j] ;  A = blockdiag-softmax_j(exp(L))
    Wbig = Wv @ (A_bd^T Wp)           [256,256]
    out  = X @ Wbig + bp

Two streaming passes over X (16.8 MB in / 16.8 MB out per core) plus a tiny
256x256 chain in between.  Each of the 8 cores processes one batch (data
parallel, no collectives).

Scheduling notes (cost-model driven):
  - DMA is the roofline: 46.6us in + 46.6us out at 360 GB/s, structurally
    serial (out depends on all of in via S -> Wbig).  Everything else must
    hide under it.
  - PE transposes use a bf16 identity as the moving operand (numerically
    exact: 1.0/0.0 are exact in bf16); transpose rate keys off the moving
    dtype -> 1.0 cycles/row instead of 1.5 for f32r.  Pass-1 PE per token
    tile = 2x107 (S matmul) + 2x53 (transpose) = 321ns < 364ns DMA pace.
  - Weights load AFTER the x stream on the same SWDGE queue so the last x
    byte (the S critical path) lands ~3us earlier; wqk is first so phase B
    isn't gated on it.  Weights land directly as f32r (casting DMA).
  - All ACT functions (Copy/Identity/Ln/Exp) live in one activation-table
    set: 1/sqrt(x) is computed as exp(-0.5*ln x), so no mid-kernel
    LoadActFuncSet (1283ns each) appears on the phase-B critical path.
  - Phase B splits chunk m=0 / m=1 work across DVE / ACT / Pool engines.
"""

import sys

if "/opt/trn_rl_repo" not in sys.path:
    sys.path.insert(0, "/opt/trn_rl_repo")

from contextlib import ExitStack

import numpy as np

import concourse.bass as bass
import concourse.tile as tile
from concourse import bacc, mybir
from concourse import bass_utils
from concourse.bass import ds, ts
from concourse.bass_interp import get_hw_module
from concourse.masks import make_identity

F32 = mybir.dt.float32
F32R = mybir.dt.float32r    # PE fast-fp32 (TF32-like, ~1.5e-4 rel); 4x faster N>=256
BF16 = mybir.dt.bfloat16
ALU = mybir.AluOpType
ACTF = mybir.ActivationFunctionType
PSUM = bass.MemorySpace.PSUM

N_CORES = 8
B, H, W, C = 8, 128, 128, 256
HEADS, DH = 8, 32
N = H * W            # 16384 tokens per batch
P = 128              # partitions / token tile
NT = N // P          # 128 token tiles
DMA_TILES = 8        # token tiles per DMA (1 MiB chunks)
NCHUNK = C // P      # 2 channel chunks


def _build_kernel(nc: bacc.Bacc):
    x_dram = nc.dram_tensor("x_in", [N, C], F32, kind="ExternalInput").ap()
    wq_dram = nc.dram_tensor("Wq", [C, C], F32, kind="ExternalInput").ap()
    wk_dram = nc.dram_tensor("Wk", [C, C], F32, kind="ExternalInput").ap()
    wv_dram = nc.dram_tensor("Wv", [C, C], F32, kind="ExternalInput").ap()
    resc_dram = nc.dram_tensor("rescale", [HEADS, 1, 1], F32, kind="ExternalInput").ap()
    wp_dram = nc.dram_tensor("Wp", [C, C], F32, kind="ExternalInput").ap()
    bp_dram = nc.dram_tensor("bp", [C], F32, kind="ExternalInput").ap()
    out_dram = nc.dram_tensor("out", [N, C], F32, kind="ExternalOutput").ap()

    with tile.TileContext(nc) as tc, ExitStack() as top:
        consts = top.enter_context(tc.tile_pool(name="consts", bufs=1))
        xt_pool = top.enter_context(tc.tile_pool(name="xt", bufs=1))
        s_pool = top.enter_context(tc.tile_pool(name="spsum", bufs=1, space=PSUM))

        # ------------- const tiles (instructions emitted inside pass-1 g==0) -------------
        identity_f = consts.tile([P, P], F32)
        identity_r = consts.tile([P, P], F32R)   # moving operand of PE transposes
        p8 = consts.tile([HEADS, C], F32)        # p8[h,c] = 1 iff c//32 == h
        p8_r = consts.tile([HEADS, C], F32R)
        bdmask = consts.tile([P, NCHUNK, C], F32)  # block-diag head mask chunks
        ones_col_f = consts.tile([P, 1], F32)
        ones_col = consts.tile([P, 1], F32R)     # [128,1] ones: column-sum matmuls
        ones_row = consts.tile([1, P], F32)      # [1,128] ones: partition broadcast
        ones_row_r = consts.tile([1, P], F32R)
        d11 = consts.tile([1, 1], F32)           # ACT table pin scratch

        # weight tiles: loaded AFTER the x stream (DMA order), directly as f32r
        wqk_r = consts.tile([P, NCHUNK, 2 * C], F32R)    # [Wq | Wk] row chunks
        wp_r = consts.tile([P, NCHUNK, C], F32R)
        wv_r = consts.tile([P, NCHUNK, C], F32R)
        wvT = consts.tile([P, NCHUNK, C], F32R)          # wvT[p,k,c] = Wv[c, 128k+p]
        bp_sb = consts.tile([1, C], F32)
        bp_r = consts.tile([1, C], F32R)
        resc_sb = consts.tile([HEADS, 1], F32)
        resc_r = consts.tile([HEADS, 1], F32R)
        rexp_sb = consts.tile([1, C], F32)       # rescale broadcast over head blocks
        wbig0 = consts.tile([P, C], F32R)
        wbig1 = consts.tile([P, C], F32R)
        wbig_l = [wbig0, wbig1]

        xT = xt_pool.tile([P, NCHUNK, N], F32R)  # X^T (f32r-rounded), from pass 1

        s_ps0 = s_pool.tile([P, C], F32, space=PSUM)
        s_ps1 = s_pool.tile([P, C], F32, space=PSUM)
        s_ps = [s_ps0, s_ps1]

        # tiny inputs first on HWDGE: ~nothing on the DMA device, needed early
        nc.sync.dma_start(resc_sb[:], resc_dram.rearrange("h a b -> h (a b)"))
        nc.sync.dma_start(bp_sb[:], bp_dram.rearrange("(a c) -> a c", a=1))

        # ---------------- pass 1: S = X^T X, and X^T via PE ----------------
        with tc.tile_pool(name="tp", bufs=5, space=PSUM) as tp_pool, tc.tile_pool(
            name="xload", bufs=4
        ) as xload:
            for g in range(NT // DMA_TILES):
                xr = xload.tile([P, DMA_TILES, C], F32R, tag="xr")
                # casting DMA: loads fp32 from HBM, rounds to f32r in-flight
                if g == 0:
                    # small first piece so PE starts sooner
                    for lo, n_t in ((0, 2), (2, 6)):
                        nc.gpsimd.dma_start(
                            xr[:, ds(lo, n_t), :],
                            x_dram[ds(lo * P, n_t * P), :].rearrange(
                                "(a p) c -> p a c", p=P
                            ),
                        )
                elif g == NT // DMA_TILES - 1:
                    # small last piece: the S tail (critical path into phase B)
                    # only waits on 2 tiles of matmul after the last byte
                    for lo, n_t in ((0, 6), (6, 2)):
                        nc.gpsimd.dma_start(
                            xr[:, ds(lo, n_t), :],
                            x_dram[ds((g * DMA_TILES + lo) * P, n_t * P), :].rearrange(
                                "(a p) c -> p a c", p=P
                            ),
                        )
                else:
                    nc.gpsimd.dma_start(
                        xr[:],
                        x_dram[ds(g * DMA_TILES * P, DMA_TILES * P), :].rearrange(
                            "(a p) c -> p a c", p=P
                        ),
                    )
                if g == 0:
                    # masks / identity (gpsimd) — behind chunk0's descriptor gen
                    make_identity(nc, identity_f[:])
                    nc.vector.tensor_copy(identity_r[:], identity_f[:])
                    nc.gpsimd.memset(p8[:], 0.0)
                    nc.gpsimd.affine_select(
                        out=p8[:].rearrange("p (b i) -> p b i", i=DH),
                        in_=p8[:].rearrange("p (b i) -> p b i", i=DH),
                        compare_op=ALU.not_equal,
                        fill=1.0,
                        base=0,
                        pattern=[[-1, HEADS], [0, DH]],
                        channel_multiplier=1,
                    )
                    nc.vector.tensor_copy(p8_r[:], p8[:])
                    nc.gpsimd.memset(bdmask[:], 0.0)
                    for r in range(NCHUNK):
                        for a2 in range(P // DH):
                            nc.gpsimd.memset(
                                bdmask[ts(a2, DH), r, ds(r * P + a2 * DH, DH)], 1.0
                            )
                    nc.gpsimd.memset(ones_col_f[:], 1.0)
                    nc.vector.tensor_copy(ones_col[:], ones_col_f[:])
                    nc.gpsimd.memset(ones_row[:], 1.0)
                    nc.vector.tensor_copy(ones_row_r[:], ones_row[:])
                    # pin the ln/exp activation-table set NOW; Copy/Identity/
                    # Ln/Exp all live in this set so it never swaps again
                    nc.scalar.activation(d11[:], ones_row[:, 0:1], ACTF.Ln)
                if g == 1:
                    # rexp[j] = rescale[head(j)] — off the critical path here
                    nc.vector.tensor_copy(resc_r[:], resc_sb[:])
                    nc.vector.tensor_copy(bp_r[:], bp_sb[:])
                    rexp_ps = tp_pool.tile([1, C], F32, space=PSUM, tag="rx", bufs=1)
                    nc.tensor.matmul(rexp_ps[:], resc_r[:], p8_r[:])
                    nc.vector.tensor_copy(rexp_sb[:], rexp_ps[:])
                for a in range(DMA_TILES):
                    t = g * DMA_TILES + a
                    x_t = xr[:, a, :]
                    first, last = t == 0, t == NT - 1
                    # both chunk transposes land in ONE psum bank (disjoint
                    # column halves); a single strided eviction then writes
                    # both xT chunks -> half the eviction ops on DVE/ACT
                    tp = tp_pool.tile([P, 2 * P], F32R, space=PSUM, tag="tp")
                    for k in range(NCHUNK):
                        nc.tensor.matmul(
                            s_ps[k][:],
                            x_t[:, ts(k, P)],
                            x_t[:],
                            start=first,
                            stop=last,
                        )
                        nc.tensor.transpose(tp[:, ts(k, P)], x_t[:, ts(k, P)], identity_r[:])
                    tp_v = tp[:].rearrange("p (k c) -> p k c", k=NCHUNK)
                    if t % 2 == 0:
                        nc.vector.tensor_copy(xT[:, :, ts(t, P)], tp_v)
                    else:
                        nc.scalar.copy(xT[:, :, ts(t, P)], tp_v)

            # weight loads ride the same SWDGE queue AFTER all x groups: they
            # transfer in the phase-B DMA-idle window instead of delaying the
            # last x byte.  wqk first (first consumer), then wp/wv.
            for k in range(NCHUNK):
                nc.gpsimd.dma_start(wqk_r[:, k, 0:C], wq_dram[ts(k, P), :])
                nc.gpsimd.dma_start(wqk_r[:, k, C : 2 * C], wk_dram[ts(k, P), :])
            for k in range(NCHUNK):
                nc.gpsimd.dma_start(wp_r[:, k, :], wp_dram[ts(k, P), :])
            for k in range(NCHUNK):
                nc.gpsimd.dma_start(wv_r[:, k, :], wv_dram[ts(k, P), :])

        # ---------------- phase B: 256x256 attention math ----------------
        # Chunked intermediates (Tile tracks deps per tensor); m=0 work goes
        # to DVE, m=1 to ACT or Pool so the two chunks pipeline in parallel.
        with tc.tile_pool(name="bwork", bufs=3, space=PSUM) as bwork, tc.tile_pool(
            name="bsmall", bufs=2, space=PSUM
        ) as bsmall, tc.tile_pool(name="bsb", bufs=1) as bsb:
            s_sbl, p12_psl, p12_sbl, gkl = [], [], [], []
            for k in range(NCHUNK):
                s_k = bsb.tile([P, C], F32R, name=f"s_sb{k}", tag="ssb", bufs=2)
                if k == 0:
                    nc.vector.tensor_copy(s_k[:], s_ps[k][:])
                else:
                    nc.scalar.copy(s_k[:], s_ps[k][:])
                s_sbl.append(s_k)

            # P12 = S @ [Wq | Wk]   (uses S symmetric: lhsT = S chunks)
            for m in range(NCHUNK):
                pp = bwork.tile([P, 2 * C], F32, space=PSUM, name=f"p12ps{m}", tag="bw", bufs=3)
                for k in range(NCHUNK):
                    nc.tensor.matmul(
                        pp[:],
                        s_sbl[k][:, ts(m, P)],
                        wqk_r[:, k, :],
                        start=(k == 0),
                        stop=(k == 1),
                    )
                p12_psl.append(pp)
            for m in range(NCHUNK):
                psb = bsb.tile([P, 2 * C], F32R, name=f"p12sb{m}", tag="p12sb", bufs=2)
                if m == 0:
                    nc.vector.tensor_copy(psb[:], p12_psl[m][:])
                else:
                    nc.scalar.copy(psb[:], p12_psl[m][:])
                p12_sbl.append(psb)

            # [G | Kgram] = Wk^T @ [P1 | P2]
            for m in range(NCHUNK):
                gg = bwork.tile([P, 2 * C], F32, space=PSUM, name=f"gkps{m}", tag="bw", bufs=3)
                for k in range(NCHUNK):
                    nc.tensor.matmul(
                        gg[:],
                        wqk_r[:, k, ds(C + m * P, P)],
                        p12_sbl[k][:],
                        start=(k == 0),
                        stop=(k == 1),
                    )
                gkl.append(gg)

            # wvT via PE transposes — emitted here so the PE does them in its
            # idle window after GK while waiting for the softmax chain; wv_r
            # has landed (~51us) by then.
            wvt_ps = bwork.tile([P, 2 * C], F32, space=PSUM, name="wvtps", tag="bw", bufs=3)
            for k in range(NCHUNK):
                for m in range(NCHUNK):
                    nc.tensor.transpose(
                        wvt_ps[:, ds((2 * k + m) * P, P)].bitcast(F32R),
                        wv_r[:, m, ts(k, P)],
                        identity_r[:],
                    )
            wvt_v = wvt_ps[:].rearrange("p (k c) -> p k c", k=NCHUNK)
            nc.vector.tensor_copy(wvT[:, 0, :], wvt_v[:, 0, :].bitcast(F32R))
            nc.scalar.copy(wvT[:, 1, :], wvt_v[:, 1, :].bitcast(F32R))

            # nk2 rows: diag of Kgram chunk m  -> per-partition [128,1]
            nk2 = bsb.tile([P, NCHUNK], F32)
            scrap0 = bsb.tile([P, P], F32)
            scrap1 = bsb.tile([P, P], F32)
            nc.vector.scalar_tensor_tensor(
                out=scrap0[:],
                in0=gkl[0][:, ds(C, P)],
                scalar=1.0,
                in1=identity_f[:],
                op0=ALU.mult,
                op1=ALU.mult,
                accum_out=nk2[:, 0:1],
            )
            nc.vector.scalar_tensor_tensor(
                out=scrap1[:],
                in0=gkl[1][:, ds(C + P, P)],
                scalar=1.0,
                in1=identity_f[:],
                op0=ALU.mult,
                op1=ALU.mult,
                accum_out=nk2[:, 1:2],
            )
            # rk = nk2^-0.5 = exp(-0.5 * ln nk2): stays inside the one table set
            nk_ln = bsb.tile([P, NCHUNK], F32)
            nc.scalar.activation(nk_ln[:], nk2[:], ACTF.Ln)
            rk = bsb.tile([P, NCHUNK], F32)
            nc.scalar.activation(rk[:], nk_ln[:], ACTF.Exp, scale=-0.5)

            # nq2[j] = sum_c Wq[c,j] P1[c,j]  -> [1, 256] via ones-matmul
            qpl = []
            for k in range(NCHUNK):
                qp = bsb.tile([P, C], F32R, name=f"qp{k}", tag="qp", bufs=2)
                eng = nc.vector
                eng.tensor_mul(
                    qp[:],
                    wqk_r[:, k, 0:C].bitcast(F32),
                    p12_sbl[k][:, 0:C].bitcast(F32),
                )
                qpl.append(qp)
            nq2_ps = bsmall.tile([1, C], F32, space=PSUM, tag="bs")
            for k in range(NCHUNK):
                nc.tensor.matmul(
                    nq2_ps[:], ones_col[:], qpl[k][:], start=(k == 0), stop=(k == 1)
                )

            # column scale: rq[j] * rescale[head(j)], rq = exp(-0.5 ln nq2)
            nq_ln = bsb.tile([1, C], F32)
            nc.scalar.activation(nq_ln[:], nq2_ps[:], ACTF.Ln)
            rq = bsb.tile([1, C], F32)
            nc.scalar.activation(rq[:], nq_ln[:], ACTF.Exp, scale=-0.5)
            colscale = bsb.tile([1, C], F32R)
            nc.vector.tensor_mul(colscale[:], rq[:], rexp_sb[:])
            csbc_ps = bsmall.tile([P, C], F32, space=PSUM, tag="bs")
            nc.tensor.matmul(csbc_ps[:], ones_row_r[:], colscale[:])
            csbc_sb = bsb.tile([P, C], F32)
            nc.scalar.copy(csbc_sb[:], csbc_ps[:])

            # logits -> exp -> masked softmax -> A (block-diagonal elsewhere 0)
            al = []
            for m in range(NCHUNK):
                sc = bsb.tile([P, C], F32, name=f"sc{m}", tag="sc", bufs=2)
                nc.vector.scalar_tensor_tensor(
                    out=sc[:],
                    in0=gkl[m][:, 0:C],
                    scalar=rk[:, m : m + 1],
                    in1=csbc_sb[:],
                    op0=ALU.mult,
                    op1=ALU.mult,
                )
                e = bsb.tile([P, C], F32, name=f"e{m}", tag="e", bufs=2)
                nc.scalar.activation(e[:], sc[:], ACTF.Exp)
                em = bsb.tile([P, C], F32, name=f"em{m}", tag="em", bufs=2)
                den = bsb.tile([P, 1], F32, name=f"den{m}", tag="den", bufs=2)
                nc.vector.scalar_tensor_tensor(
                    out=em[:],
                    in0=e[:],
                    scalar=1.0,
                    in1=bdmask[:, m, :],
                    op0=ALU.mult,
                    op1=ALU.mult,
                    accum_out=den[:],
                )
                rden = bsb.tile([P, 1], F32, name=f"rden{m}", tag="rden", bufs=2)
                nc.vector.reciprocal(rden[:], den[:])
                a_m = bsb.tile([P, C], F32R, name=f"a{m}", tag="a", bufs=2)
                if m == 0:
                    nc.vector.tensor_scalar_mul(a_m[:], em[:], rden[:])
                else:
                    nc.scalar.mul(a_m[:], em[:], rden[:])
                al.append(a_m)

            # T1 = A_bd^T @ Wp  (lhsT = A_bd chunks directly)
            t1_sbl = []
            for m in range(NCHUNK):
                t1p = bwork.tile([P, C], F32, space=PSUM, name=f"t1ps{m}", tag="bw", bufs=3)
                for k in range(NCHUNK):
                    nc.tensor.matmul(
                        t1p[:],
                        al[k][:, ts(m, P)],
                        wp_r[:, k, :],
                        start=(k == 0),
                        stop=(k == 1),
                    )
                t1s = bsb.tile([P, C], F32R, name=f"t1sb{m}", tag="t1sb", bufs=2)
                if m == 0:
                    nc.vector.tensor_copy(t1s[:], t1p[:])
                else:
                    nc.scalar.copy(t1s[:], t1p[:])
                t1_sbl.append(t1s)

            # Wbig = Wv @ T1  (lhsT = Wv^T chunks)
            for m in range(NCHUNK):
                wbp = bwork.tile([P, C], F32, space=PSUM, name=f"wbps{m}", tag="bw", bufs=3)
                for k in range(NCHUNK):
                    nc.tensor.matmul(
                        wbp[:],
                        wvT[:, k, ts(m, P)],
                        t1_sbl[k][:],
                        start=(k == 0),
                        stop=(k == 1),
                    )
                if m == 0:
                    nc.vector.tensor_copy(wbig_l[m][:], wbp[:])
                else:
                    nc.scalar.copy(wbig_l[m][:], wbp[:])

        # ---------------- pass 2: out = X @ Wbig + bp ----------------
        # First groups are small so the store DMA starts ASAP after Wbig;
        # steady state is DMA-bound (2913ns per 8-tile store vs 2568ns PE).
        group_sizes = [2, 2, 4] + [8] * 15
        assert sum(group_sizes) == NT
        with tc.tile_pool(name="ops", bufs=6, space=PSUM) as ops, tc.tile_pool(
            name="outb", bufs=3
        ) as outb:
            t0 = 0
            for gi, gsz in enumerate(group_sizes):
                ob = outb.tile([P, gsz, C], F32, tag=f"ob{gsz}")
                for a2 in range(gsz // 2):
                    # two tiles' outputs share one PSUM bank (sequential
                    # accumulation groups in disjoint halves); one strided
                    # eviction writes both -> half the DVE/ACT ops
                    o_ps = ops.tile([P, 2 * C], F32, space=PSUM, tag="o")
                    for h2 in range(2):
                        t = t0 + a2 * 2 + h2
                        for k in range(NCHUNK):
                            nc.tensor.matmul(
                                o_ps[:, ts(h2, C)],
                                xT[:, k, ts(t, P)],
                                wbig_l[k][:],
                                start=(k == 0),
                                stop=False,
                            )
                        nc.tensor.matmul(
                            o_ps[:, ts(h2, C)],
                            ones_row_r[:],
                            bp_r[:],
                            start=False,
                            stop=True,
                        )
                    o_v = o_ps[:].rearrange("p (h c) -> p h c", h=2)
                    if a2 % 2 == 0:
                        nc.vector.tensor_copy(ob[:, ds(a2 * 2, 2), :], o_v)
                    else:
                        nc.scalar.copy(ob[:, ds(a2 * 2, 2), :], o_v)
                if gi == len(group_sizes) - 1:
                    half = gsz // 2
                    for h2 in range(2):
                        nc.sync.dma_start(
                            out_dram[ds((t0 + h2 * half) * P, half * P), :].rearrange(
                                "(a p) c -> p a c", p=P
                            ),
                            ob[:, ts(h2, half), :],
                        )
                else:
                    nc.sync.dma_start(
                        out_dram[ds(t0 * P, gsz * P), :].rearrange(
                            "(a p) c -> p a c", p=P
                        ),
                        ob[:],
                    )
                t0 += gsz

    return nc


_NC_CACHE = None


def _get_nc():
    global _NC_CACHE
    if _NC_CACHE is None:
        nc = bacc.Bacc(
            "TRN2",
            target_bir_lowering=False,
            debug=False,
            enable_asserts=False,
            num_devices=N_CORES,
        )
        _build_kernel(nc)
        nc.compile()
        nc.m = get_hw_module(nc.m)
        _NC_CACHE = nc
    return _NC_CACHE


def _make_in_maps(x_in, Wq, Wk, Wv, rescale, Wp, bp):
    x_in = np.ascontiguousarray(np.asarray(x_in, dtype=np.float32))
    maps = []
    for core in range(N_CORES):
        maps.append(
            {
                "x_in": x_in[core].reshape(N, C),
                "Wq": np.asarray(Wq, np.float32),
                "Wk": np.asarray(Wk, np.float32),
                "Wv": np.asarray(Wv, np.float32),
                "rescale": np.asarray(rescale, np.float32),
                "Wp": np.asarray(Wp, np.float32),
                "bp": np.asarray(bp, np.float32),
            }
        )
    return maps


def run_on_hw(inputs: dict, trace: bool = False, tmpdir: str | None = None):
    """Returns (full_output [8,128,128,256] f32, BassKernelResults)."""
    nc = _get_nc()
    in_maps = _make_in_maps(**inputs)
    res = bass_utils.run_bass_kernel_spmd(
        nc, in_maps, core_ids=list(range(N_CORES)), trace=trace, tmpdir=tmpdir
    )
    out = np.stack([res.results[c]["out"].reshape(H, W, C) for c in range(N_CORES)])
    return out.astype(np.float32), res


def kernel(x_in, Wq, Wk, Wv, rescale, Wp, bp) -> np.ndarray:
    out, _ = run_on_hw(
        dict(x_in=x_in, Wq=Wq, Wk=Wk, Wv=Wv, rescale=rescale, Wp=Wp, bp=bp)
    )
    return out


# revision 54
# speedup vs baseline: 1.0316x; 1.0302x over previous
"""Trainium2 Bass kernel for channel-wise ("transposed") attention.

Reference computation (per batch b, X = x_in[b] reshaped [N=16384, C=256]):
    Q = X Wq ; K = X Wk ; V = X Wv            (columns l2-normalized over tokens for Q,K)
    attn[h,i,j] = softmax_j( qhat_i . khat_j * rescale[h] )   (32x32 per head)
    out = (A_bd @ V^T)^T Wp + bp

Algebraic reduction (validated vs reference):
    S    = X^T X                      [256,256]   (only pass-1 reduction needed)
    P1   = S Wq ; P2 = S Wk
    G    = Wk^T P1                    (raw cross-gram K^T Q)
    nq2  = diag(Wq^T P1) ; nk2 = diag(Wk^T P2)
    L    = G * rk[i] * (rq*rescale_expanded)[j] ;  A = blockdiag-softmax_j(exp(L))
    Wbig = Wv @ (A_bd^T Wp)           [256,256]
    out  = X @ Wbig + bp

Two streaming passes over X (16.8 MB in / 16.8 MB out per core) plus a tiny
256x256 chain in between.  Each of the 8 cores processes one batch (data
parallel, no collectives).

Scheduling notes (cost-model driven; DMA at 360 GB/s is the roofline and the
two passes are structurally serial through S -> Wbig):
  - First two token tiles load as plain fp32 on the HWDGE (sync) queue: it
    has ~1.4us init vs ~2.0us for SWDGE, and fp32 matmuls for just those
    tiles cost nothing while the PE is otherwise idle.
  - All other x tiles ride one SWDGE (gpsimd) queue as casting f32->f32r
    DMAs; weights are queued on the same ring AFTER the x stream so the
    last x byte (the S critical path) lands ~3us earlier.  The descriptor
    ring is enlarged (dynamic_dma_scratch_size) so weight descriptor-gen
    runs ahead instead of being throttled to the transfer pace.
  - Pool-engine const building is emitted at g==2 so it does not delay
    group descriptor generation.
  - PE pass-1 pace is 374ns/tile vs 364ns/tile DMA: transposes for tiles
    0-1 and 112-127 are deferred into pass-2's PE slack so S finishes
    right behind the last DMA byte instead of ~5us later.
  - 1/sqrt(x) is computed as exp(-0.5*ln x): Copy/Identity/Ln/Exp all live
    in the one activation-table set `natural_log_exp_and_others`, and two
    dummy activations at the start force its load off the critical path.
  - Phase B splits chunk m=0 / m=1 eviction work across DVE / ACT.
"""

import sys

if "/opt/trn_rl_repo" not in sys.path:
    sys.path.insert(0, "/opt/trn_rl_repo")

from contextlib import ExitStack

import numpy as np

import concourse.bass as bass
import concourse.tile as tile
from concourse import bacc, mybir
from concourse import bass_utils
from concourse.bass import ds, ts
from concourse.bass_interp import get_hw_module
from concourse.masks import make_identity

F32 = mybir.dt.float32
F32R = mybir.dt.float32r    # PE fast-fp32 (TF32-like, ~1.5e-4 rel); 4x faster N>=256
BF16 = mybir.dt.bfloat16    # X^T / Wbig storage: halves SBUF, ~2e-3 rel (gate 2e-2)
ALU = mybir.AluOpType
ACTF = mybir.ActivationFunctionType
PSUM = bass.MemorySpace.PSUM

N_CORES = 8
B, H, W, C = 8, 128, 128, 256
HEADS, DH = 8, 32
N = H * W            # 16384 tokens per batch
P = 128              # partitions / token tile
NT = N // P          # 128 token tiles
NCHUNK = C // P      # 2 channel chunks

# token-tile groups for the x load stream: (start, ntiles)
# small first piece so PE starts early; 8-tile steady state; small tail so
# the final S matmuls wait on as little data as possible.
X_GROUPS = [(0, 2), (2, 6)] + [(8 + 8 * i, 8) for i in range(14)] + [
    (120, 6), (126, 2)
]
# Transposes deferred into pass 2: balances PE below the DMA pace in BOTH
# passes (pass1 374->~330ns/tile avg, pass2 321->~360 incl deferred work).
# Deferred tiles must still be SBUF-resident in pass 2: 96-127 sit in the
# last bufs=6 xload allocations.
DEFER_T = set(range(96, 128))


def _build_kernel(nc: bacc.Bacc):
    x_dram = nc.dram_tensor("x_in", [N, C], F32, kind="ExternalInput").ap()
    wq_dram = nc.dram_tensor("Wq", [C, C], F32, kind="ExternalInput").ap()
    wk_dram = nc.dram_tensor("Wk", [C, C], F32, kind="ExternalInput").ap()
    wv_dram = nc.dram_tensor("Wv", [C, C], F32, kind="ExternalInput").ap()
    resc_dram = nc.dram_tensor("rescale", [HEADS, 1, 1], F32, kind="ExternalInput").ap()
    wp_dram = nc.dram_tensor("Wp", [C, C], F32, kind="ExternalInput").ap()
    bp_dram = nc.dram_tensor("bp", [C], F32, kind="ExternalInput").ap()
    out_dram = nc.dram_tensor("out", [N, C], F32, kind="ExternalOutput").ap()

    with tile.TileContext(nc) as tc, ExitStack() as top:
        consts = top.enter_context(tc.tile_pool(name="consts", bufs=1))
        xt_pool = top.enter_context(tc.tile_pool(name="xt", bufs=1))
        # xload stays open through pass 2: the deferred-transpose tiles
        # (112-127) must stay resident until transposed there.  bufs=6:
        # each group's DMA has a WAR dependency on the reads of the group
        # 6 allocations earlier, so descriptor-gen runs well ahead of the
        # PE and the weight gens (queued after x) finish early too.
        xload = top.enter_context(tc.tile_pool(name="xload", bufs=6))

        # ------------- const tiles -------------
        identity_f = consts.tile([P, P], F32)
        identity_r = consts.tile([P, P], F32R)
        p8 = consts.tile([HEADS, C], F32)        # p8[h,c] = 1 iff c//32 == h
        p8_r = consts.tile([HEADS, C], F32R)
        bdmask = consts.tile([P, NCHUNK, C], F32)  # block-diag head mask chunks
        ones_col_f = consts.tile([P, 1], F32)
        ones_col = consts.tile([P, 1], F32R)     # [128,1] ones: column-sum matmuls
        ones_row = consts.tile([1, P], F32)      # [1,128] ones: partition broadcast
        ones_row_r = consts.tile([1, P], F32R)
        d11 = consts.tile([1, 1], F32)           # ACT table pin scratch

        # weight tiles: loaded AFTER the x stream (same SWDGE ring), as f32r
        wqk_r = consts.tile([P, NCHUNK, 2 * C], F32R)    # [Wq | Wk] row chunks
        wp_r = consts.tile([P, NCHUNK, C], F32R)
        wv_r = consts.tile([P, NCHUNK, C], F32R)
        wvT = consts.tile([P, NCHUNK, C], F32R)          # wvT[p,k,c] = Wv[c, 128k+p]
        bp_sb = consts.tile([1, C], F32)
        bp_r = consts.tile([1, C], F32R)
        resc_sb = consts.tile([HEADS, 1], F32)
        resc_r = consts.tile([HEADS, 1], F32R)
        rexp_sb = consts.tile([1, C], F32)       # rescale broadcast over head blocks
        wbig0 = consts.tile([P, C], BF16)
        wbig1 = consts.tile([P, C], BF16)
        wbig_l = [wbig0, wbig1]

        xT = xt_pool.tile([P, NCHUNK, N], BF16)  # X^T (bf16-rounded), from pass 1

        # tiny inputs on HWDGE: DMA device cost ~nothing
        nc.sync.dma_start(resc_sb[:], resc_dram.rearrange("h a b -> h (a b)"))
        nc.sync.dma_start(bp_sb[:], bp_dram.rearrange("(a c) -> a c", a=1))
        # pin the ln table set up front via a dummy (rescale is ones, so Ln
        # sees 1.0).  The one ln->exp table switch in phase B then picks the
        # combined natural_log_exp set; Copy/Identity are in every set.
        nc.scalar.activation(d11[:], resc_sb[0:1, 0:1], ACTF.Ln)

        with tc.tile_pool(name="spsum", bufs=1, space=PSUM) as s_pool:
            s_ps0 = s_pool.tile([P, C], F32, space=PSUM)
            s_ps1 = s_pool.tile([P, C], F32, space=PSUM)
            s_ps = [s_ps0, s_ps1]

            # ---------------- pass 1: S = X^T X, and X^T via PE ----------------
            with tc.tile_pool(name="tp", bufs=4, space=PSUM) as tp_pool:
                xr_tiles = {}
                for gi, (t0, n_t) in enumerate(X_GROUPS):
                    xr_full = xload.tile([P, 8, C], F32R, tag="xr")
                    xr = xr_full[:, ds(0, n_t), :]
                    xr_tiles[gi] = xr
                    nc.gpsimd.dma_start(
                        xr,
                        x_dram[ds(t0 * P, n_t * P), :].rearrange(
                            "(a p) c -> p a c", p=P
                        ),
                    )
                    if gi == 0:
                        # identity must exist before group-0's transposes;
                        # memset/copy on DVE, affine_select is Pool-only
                        nc.vector.memset(identity_f[:], 0.0)
                        nc.gpsimd.affine_select(
                            out=identity_f[:],
                            in_=identity_f[:],
                            compare_op=ALU.not_equal,
                            fill=1.0,
                            base=0,
                            pattern=[[-1, P]],
                            channel_multiplier=1,
                        )
                        nc.vector.tensor_copy(identity_r[:], identity_f[:])
                    if gi == 1:
                        # remaining consts: DVE memsets/copies + one Pool
                        # affine, emitted behind group-1's descriptor gen
                        nc.vector.memset(p8[:], 0.0)
                        nc.gpsimd.affine_select(
                            out=p8[:].rearrange("p (b i) -> p b i", i=DH),
                            in_=p8[:].rearrange("p (b i) -> p b i", i=DH),
                            compare_op=ALU.not_equal,
                            fill=1.0,
                            base=0,
                            pattern=[[-1, HEADS], [0, DH]],
                            channel_multiplier=1,
                        )
                        nc.vector.tensor_copy(p8_r[:], p8[:])
                        nc.vector.memset(bdmask[:], 0.0)
                        for r in range(NCHUNK):
                            for a2 in range(P // DH):
                                nc.vector.memset(
                                    bdmask[ts(a2, DH), r, ds(r * P + a2 * DH, DH)], 1.0
                                )
                        nc.vector.memset(ones_col_f[:], 1.0)
                        nc.vector.tensor_copy(ones_col[:], ones_col_f[:])
                        nc.vector.memset(ones_row[:], 1.0)
                        nc.vector.tensor_copy(ones_row_r[:], ones_row[:])
                        nc.vector.tensor_copy(resc_r[:], resc_sb[:])
                        nc.vector.tensor_copy(bp_r[:], bp_sb[:])
                    if gi == 3:
                        # rexp[j] = rescale[head(j)] — off the critical path
                        rexp_ps = tp_pool.tile([1, C], F32, space=PSUM, tag="rx", bufs=1)
                        nc.tensor.matmul(rexp_ps[:], resc_r[:], p8_r[:])
                        nc.vector.tensor_copy(rexp_sb[:], rexp_ps[:])
                    for a in range(n_t):
                        t = t0 + a
                        x_t = xr[:, a, :]
                        last = t == NT - 1
                        for k in range(NCHUNK):
                            nc.tensor.matmul(
                                s_ps[k][:],
                                x_t[:, ts(k, P)],
                                x_t[:],
                                start=(t == 0),
                                stop=last,
                            )
                        if t in DEFER_T:
                            continue
                        # both chunk transposes share ONE psum bank; a single
                        # strided eviction writes both xT chunks
                        tp = tp_pool.tile([P, 2 * P], F32R, space=PSUM, tag="tp")
                        for k in range(NCHUNK):
                            nc.tensor.transpose(
                                tp[:, ts(k, P)], x_t[:, ts(k, P)], identity_r[:]
                            )
                        tp_v = tp[:].rearrange("p (k c) -> p k c", k=NCHUNK)
                        if t % 2 == 0:
                            nc.vector.tensor_copy(xT[:, :, ts(t, P)], tp_v)
                        else:
                            nc.scalar.copy(xT[:, :, ts(t, P)], tp_v)
                    if gi == 0:
                        # PE clock-ramp keep-alive: the cost model drops to
                        # half clock after any idle until 3us of continuous
                        # busy.  These dep-free dummy transposes keep PE busy
                        # across the gap until group 1 lands, so the ramp
                        # completes ~2us sooner.
                        warm = tp_pool.tile([P, P], F32R, space=PSUM, tag="warm", bufs=1)
                        for _ in range(8):
                            nc.tensor.transpose(warm[:], identity_r[:], identity_r[:])

                # weight loads ride the same SWDGE ring AFTER all x groups:
                # they transfer in the phase-B DMA-idle window.  One DMA per
                # weight matrix (descriptor-gen costs ~1us each on Pool);
                # wqk first since P12 is the first consumer.
                nc.gpsimd.dma_start(
                    wqk_r[:, :, 0:C], wq_dram.rearrange("(k p) c -> p k c", p=P)
                )
                nc.gpsimd.dma_start(
                    wqk_r[:, :, C : 2 * C],
                    wk_dram.rearrange("(k p) c -> p k c", p=P),
                )
                nc.gpsimd.dma_start(
                    wp_r[:], wp_dram.rearrange("(k p) c -> p k c", p=P)
                )
                nc.gpsimd.dma_start(
                    wv_r[:], wv_dram.rearrange("(k p) c -> p k c", p=P)
                )

            # ---------------- phase B: 256x256 attention math ----------------
            # Chunked intermediates; m=0 evictions/stt on DVE, m=1 on ACT.
            with tc.tile_pool(name="bwork", bufs=3, space=PSUM) as bwork, tc.tile_pool(
                name="bsmall", bufs=2, space=PSUM
            ) as bsmall, tc.tile_pool(name="bsb", bufs=1) as bsb:
                s_sbl, p12_psl, p12_sbl, gkl = [], [], [], []
                for k in range(NCHUNK):
                    s_k = bsb.tile([P, C], F32R, name=f"s_sb{k}", tag="ssb", bufs=2)
                    if k == 0:
                        nc.vector.tensor_copy(s_k[:], s_ps[k][:])
                    else:
                        nc.scalar.copy(s_k[:], s_ps[k][:])
                    s_sbl.append(s_k)

                # P12 = S @ [Wq | Wk]   (uses S symmetric: lhsT = S chunks)
                for m in range(NCHUNK):
                    pp = bwork.tile(
                        [P, 2 * C], F32, space=PSUM, name=f"p12ps{m}", tag="bw", bufs=3
                    )
                    for k in range(NCHUNK):
                        nc.tensor.matmul(
                            pp[:],
                            s_sbl[k][:, ts(m, P)],
                            wqk_r[:, k, :],
                            start=(k == 0),
                            stop=(k == 1),
                        )
                    p12_psl.append(pp)
                for m in range(NCHUNK):
                    psb = bsb.tile(
                        [P, 2 * C], F32R, name=f"p12sb{m}", tag="p12sb", bufs=2
                    )
                    if m == 0:
                        nc.vector.tensor_copy(psb[:], p12_psl[m][:])
                    else:
                        nc.scalar.copy(psb[:], p12_psl[m][:])
                    p12_sbl.append(psb)

                # [G | Kgram] = Wk^T @ [P1 | P2]
                for m in range(NCHUNK):
                    gg = bwork.tile(
                        [P, 2 * C], F32, space=PSUM, name=f"gkps{m}", tag="bw", bufs=3
                    )
                    for k in range(NCHUNK):
                        nc.tensor.matmul(
                            gg[:],
                            wqk_r[:, k, ds(C + m * P, P)],
                            p12_sbl[k][:],
                            start=(k == 0),
                            stop=(k == 1),
                        )
                    gkl.append(gg)

                # wvT via PE transposes — PE idle window after GK; wv_r has
                # landed by now (weights follow the x stream immediately)
                wvt_ps = bwork.tile(
                    [P, 2 * C], F32, space=PSUM, name="wvtps", tag="bw", bufs=3
                )
                for k in range(NCHUNK):
                    for m in range(NCHUNK):
                        nc.tensor.transpose(
                            wvt_ps[:, ds((2 * k + m) * P, P)].bitcast(F32R),
                            wv_r[:, m, ts(k, P)],
                            identity_r[:],
                        )
                wvt_v = wvt_ps[:].rearrange("p (k c) -> p k c", k=NCHUNK)
                nc.vector.tensor_copy(wvT[:, 0, :], wvt_v[:, 0, :].bitcast(F32R))
                nc.scalar.copy(wvT[:, 1, :], wvt_v[:, 1, :].bitcast(F32R))

                # nq2[j] = sum_c Wq[c,j] P1[c,j]  -> [1, 256] via ones-matmul
                qpl = []
                for k in range(NCHUNK):
                    qp = bsb.tile([P, C], F32R, name=f"qp{k}", tag="qp", bufs=2)
                    nc.vector.tensor_mul(
                        qp[:],
                        wqk_r[:, k, 0:C].bitcast(F32),
                        p12_sbl[k][:, 0:C].bitcast(F32),
                    )
                    qpl.append(qp)
                nq2_ps = bsmall.tile([1, C], F32, space=PSUM, tag="bs")
                for k in range(NCHUNK):
                    nc.tensor.matmul(
                        nq2_ps[:], ones_col[:], qpl[k][:], start=(k == 0), stop=(k == 1)
                    )

                # nk2 rows: diag of Kgram chunk m  -> per-partition [128,1]
                nk2 = bsb.tile([P, NCHUNK], F32)
                scrap0 = bsb.tile([P, P], F32)
                scrap1 = bsb.tile([P, P], F32)
                for m, scrap in ((0, scrap0), (1, scrap1)):
                    nc.vector.scalar_tensor_tensor(
                        out=scrap[:],
                        in0=gkl[m][:, ds(C + m * P, P)],
                        scalar=1.0,
                        in1=identity_f[:],
                        op0=ALU.mult,
                        op1=ALU.mult,
                        accum_out=nk2[:, m : m + 1],
                    )
                # x^-0.5 = exp(-0.5 ln x): the four Ln/Exp ops run
                # back-to-back on ACT (one table set, no swaps)
                nk_ln = bsb.tile([P, NCHUNK], F32)
                nc.scalar.activation(nk_ln[:], nk2[:], ACTF.Ln)
                nq_ln = bsb.tile([1, C], F32)
                nc.scalar.activation(nq_ln[:], nq2_ps[:], ACTF.Ln)
                rk = bsb.tile([P, NCHUNK], F32)
                nc.scalar.activation(rk[:], nk_ln[:], ACTF.Exp, scale=-0.5)
                rq = bsb.tile([1, C], F32)
                nc.scalar.activation(rq[:], nq_ln[:], ACTF.Exp, scale=-0.5)
                colscale = bsb.tile([1, C], F32R)
                nc.vector.tensor_mul(colscale[:], rq[:], rexp_sb[:])
                csbc_ps = bsmall.tile([P, C], F32, space=PSUM, tag="bs")
                nc.tensor.matmul(csbc_ps[:], ones_row_r[:], colscale[:])
                csbc_sb = bsb.tile([P, C], F32)
                nc.scalar.copy(csbc_sb[:], csbc_ps[:])

                # logits -> exp -> masked softmax -> A (block-diag elsewhere 0)
                al = []
                for m in range(NCHUNK):
                    sc = bsb.tile([P, C], F32, name=f"sc{m}", tag="sc", bufs=2)
                    nc.vector.scalar_tensor_tensor(
                        out=sc[:],
                        in0=gkl[m][:, 0:C],
                        scalar=rk[:, m : m + 1],
                        in1=csbc_sb[:],
                        op0=ALU.mult,
                        op1=ALU.mult,
                    )
                    e = bsb.tile([P, C], F32, name=f"e{m}", tag="e", bufs=2)
                    nc.scalar.activation(e[:], sc[:], ACTF.Exp)
                    em = bsb.tile([P, C], F32, name=f"em{m}", tag="em", bufs=2)
                    den = bsb.tile([P, 1], F32, name=f"den{m}", tag="den", bufs=2)
                    nc.vector.scalar_tensor_tensor(
                        out=em[:],
                        in0=e[:],
                        scalar=1.0,
                        in1=bdmask[:, m, :],
                        op0=ALU.mult,
                        op1=ALU.mult,
                        accum_out=den[:],
                    )
                    rden = bsb.tile([P, 1], F32, name=f"rden{m}", tag="rden", bufs=2)
                    nc.vector.reciprocal(rden[:], den[:])
                    a_m = bsb.tile([P, C], F32R, name=f"a{m}", tag="a", bufs=2)
                    if m == 0:
                        nc.vector.tensor_scalar_mul(a_m[:], em[:], rden[:])
                    else:
                        nc.scalar.mul(a_m[:], em[:], rden[:])
                    al.append(a_m)

                # T1 = A_bd^T @ Wp  (lhsT = A_bd chunks directly)
                t1_sbl = []
                for m in range(NCHUNK):
                    t1p = bwork.tile(
                        [P, C], F32, space=PSUM, name=f"t1ps{m}", tag="bw", bufs=3
                    )
                    for k in range(NCHUNK):
                        nc.tensor.matmul(
                            t1p[:],
                            al[k][:, ts(m, P)],
                            wp_r[:, k, :],
                            start=(k == 0),
                            stop=(k == 1),
                        )
                    t1s = bsb.tile([P, C], F32R, name=f"t1sb{m}", tag="t1sb", bufs=2)
                    if m == 0:
                        nc.vector.tensor_copy(t1s[:], t1p[:])
                    else:
                        nc.scalar.copy(t1s[:], t1p[:])
                    t1_sbl.append(t1s)

                # Wbig = Wv @ T1  (lhsT = Wv^T chunks)
                for m in range(NCHUNK):
                    wbp = bwork.tile(
                        [P, C], F32, space=PSUM, name=f"wbps{m}", tag="bw", bufs=3
                    )
                    for k in range(NCHUNK):
                        nc.tensor.matmul(
                            wbp[:],
                            wvT[:, k, ts(m, P)],
                            t1_sbl[k][:],
                            start=(k == 0),
                            stop=(k == 1),
                        )
                    if m == 0:
                        nc.vector.tensor_copy(wbig_l[m][:], wbp[:])
                    else:
                        nc.scalar.copy(wbig_l[m][:], wbp[:])

        # ---------------- pass 2: out = X @ Wbig + bp ----------------
        # First groups are tiny so the store DMA starts ASAP after Wbig;
        # steady state is DMA-bound (2913ns per 8-tile store vs ~2670ns PE
        # including the deferred transposes it absorbs here).
        group_sizes = [2, 2, 4] + [8] * 15
        assert sum(group_sizes) == NT
        deferred = sorted(DEFER_T)
        with tc.tile_pool(name="ops", bufs=5, space=PSUM) as ops, tc.tile_pool(
            name="outb", bufs=3
        ) as outb, tc.tile_pool(name="tp2", bufs=3, space=PSUM) as tp2:
            t0 = 0
            pair_i = 0
            for gi, gsz in enumerate(group_sizes):
                ob_full = outb.tile([P, 8, C], F32, tag="ob")
                ob = ob_full[:, ds(0, gsz), :]
                npair = (gsz + 1) // 2
                for a2 in range(npair):
                    w2 = min(2, gsz - a2 * 2)
                    # two tiles' outputs share one PSUM bank; one strided
                    # eviction writes both
                    o_ps = ops.tile([P, w2 * C], F32, space=PSUM, tag="o")
                    for h2 in range(w2):
                        t = t0 + a2 * 2 + h2
                        for k in range(NCHUNK):
                            nc.tensor.matmul(
                                o_ps[:, ts(h2, C)],
                                xT[:, k, ts(t, P)],
                                wbig_l[k][:],
                                start=(k == 0),
                                stop=False,
                            )
                        nc.tensor.matmul(
                            o_ps[:, ts(h2, C)],
                            ones_row_r[:],
                            bp_r[:],
                            start=False,
                            stop=True,
                        )
                    o_v = o_ps[:].rearrange("p (h c) -> p h c", h=w2)
                    if pair_i % 2 == 0:
                        nc.vector.tensor_copy(ob[:, ds(a2 * 2, w2), :], o_v)
                    else:
                        nc.scalar.copy(ob[:, ds(a2 * 2, w2), :], o_v)
                    pair_i += 1
                if gi == len(group_sizes) - 1:
                    half = gsz // 2
                    for h2 in range(2):
                        nc.sync.dma_start(
                            out_dram[ds((t0 + h2 * half) * P, half * P), :].rearrange(
                                "(a p) c -> p a c", p=P
                            ),
                            ob[:, ts(h2, half), :],
                        )
                else:
                    nc.sync.dma_start(
                        out_dram[ds(t0 * P, gsz * P), :].rearrange(
                            "(a p) c -> p a c", p=P
                        ),
                        ob[:],
                    )
                t0 += gsz
                # deferred transposes: fill pass-2 PE slack once the store
                # pipeline is rolling (not during the small lead-in groups)
                n_emit = 3 if 2 <= gi < 14 else 0
                for _ in range(n_emit):
                    if not deferred:
                        break
                    t = deferred.pop(0)
                    g_idx, lo = next(
                        (i, t - s) for i, (s, n_t) in enumerate(X_GROUPS)
                        if s <= t < s + n_t
                    )
                    src = xr_tiles[g_idx][:, lo, :]
                    tpd = tp2.tile([P, 2 * P], F32R, space=PSUM, tag="tr", bufs=3)
                    ident = identity_r
                    for k in range(NCHUNK):
                        nc.tensor.transpose(
                            tpd[:, ts(k, P)], src[:, ts(k, P)], ident[:]
                        )
                    tpd_v = tpd[:].rearrange("p (k c) -> p k c", k=NCHUNK)
                    if t % 2 == 0:
                        nc.vector.tensor_copy(xT[:, :, ts(t, P)], tpd_v)
                    else:
                        nc.scalar.copy(xT[:, :, ts(t, P)], tpd_v)

    return nc


_NC_CACHE = None


def _get_nc():
    global _NC_CACHE
    if _NC_CACHE is None:
        nc = bacc.Bacc(
            "TRN2",
            target_bir_lowering=False,
            debug=False,
            enable_asserts=False,
            num_devices=N_CORES,
            dynamic_dma_scratch_size=16384,
        )
        _build_kernel(nc)
        nc.compile()
        nc.m = get_hw_module(nc.m)
        _NC_CACHE = nc
    return _NC_CACHE


def _make_in_maps(x_in, Wq, Wk, Wv, rescale, Wp, bp):
    x_in = np.ascontiguousarray(np.asarray(x_in, dtype=np.float32))
    maps = []
    for core in range(N_CORES):
        maps.append(
            {
                "x_in": x_in[core].reshape(N, C),
                "Wq": np.asarray(Wq, np.float32),
                "Wk": np.asarray(Wk, np.float32),
                "Wv": np.asarray(Wv, np.float32),
                "rescale": np.asarray(rescale, np.float32),
                "Wp": np.asarray(Wp, np.float32),
                "bp": np.asarray(bp, np.float32),
            }
        )
    return maps


def run_on_hw(inputs: dict, trace: bool = False, tmpdir: str | None = None):
    """Returns (full_output [8,128,128,256] f32, BassKernelResults)."""
    nc = _get_nc()
    in_maps = _make_in_maps(**inputs)
    res = bass_utils.run_bass_kernel_spmd(
        nc, in_maps, core_ids=list(range(N_CORES)), trace=trace, tmpdir=tmpdir
    )
    out = np.stack([res.results[c]["out"].reshape(H, W, C) for c in range(N_CORES)])
    return out.astype(np.float32), res


def kernel(x_in, Wq, Wk, Wv, rescale, Wp, bp) -> np.ndarray:
    out, _ = run_on_hw(
        dict(x_in=x_in, Wq=Wq, Wk=Wk, Wv=Wv, rescale=rescale, Wp=Wp, bp=bp)
    )
    return out
